# revision 13
# baseline (speedup 1.0000x reference)
"""Trainium2 Bass kernel for a talking-heads MHSA block.

Reference computation (B=4, P=2048, D=512, H=8, DF=64, fp32):
    q = (x @ Wq) / sqrt(DF);  k = x @ Wk;  v = x @ Wv      (per-head reshape)
    attn[b,h]   = q_h k_h^T
    attn2[b,g]  = sum_h Wtalk[g,h] attn[b,h]               (talking heads)
    P           = softmax(attn2 + bias, axis=-1)
    out         = concat_g(P_g v_g) @ Wo

Sharding: 8 cores, data-parallel: core c -> batch b=c//2, query-half s=c%2
(1024 query rows, all heads, full 2048 keys). No collectives.

Per-core algorithm (bf16 matmuls, fp32 logits, zero on-chip transposes):
  - host pre-transposes x -> x^T and the bias slice -> bias^T[g, q, p]
  - talking-heads mix is folded into QK: S_mixed[g] = (Wtalk[g,h]/8 * Q)
    contracted over all 512 features against K -> one dense 512-deep matmul
  - S^T[q, p] accumulates in PSUM fp32; DVE adds fp32 bias^T in place; ACT
    computes exp -> bf16 probabilities (no max-subtraction: logits are
    bounded ~+-7, mathematically identical)
  - AV uses exp(S^T) directly as the bf16 moving operand; a ones-column in
    V' produces the softmax denominators in PSUM partition 64 for free
  - normalization is applied after AV (linear), then the output projection
    consumes out^T as lhsT directly.
"""
import sys
from contextlib import ExitStack

import numpy as np

if "/opt/trn_rl_repo" not in sys.path:
    sys.path.insert(0, "/opt/trn_rl_repo")

B, P, D = 4, 2048, 512
H, DF = 8, 64
G = H                 # output head groups
PH = P // 2           # query rows per core
DC = D // 128         # 4 contraction chunks for d
EC = (H * DF) // 128  # 4 chunks for e = (h, df)
QC = P // 128         # 16 key chunks
VW = DF + 1           # V' width per group: 64 cols of V + ones column
N_CORES = 8

_CACHE = {}
LAST_RESULTS = None


def _build_program():
    import concourse.mybir as mybir
    import concourse.tile as tile
    from concourse import bacc

    f32 = mybir.dt.float32
    bf16 = mybir.dt.bfloat16
    ACT = mybir.ActivationFunctionType

    nc = bacc.Bacc("TRN2", target_bir_lowering=False, debug=False)
    xt = nc.dram_tensor("xt", [D, P], f32, kind="ExternalInput").ap()
    xqt = nc.dram_tensor("xqt", [D, PH], f32, kind="ExternalInput").ap()
    biast = nc.dram_tensor("biast", [G, P, PH], bf16, kind="ExternalInput").ap()
    wq = nc.dram_tensor("wq", [D, H * DF], f32, kind="ExternalInput").ap()
    wk = nc.dram_tensor("wk", [D, H * DF], f32, kind="ExternalInput").ap()
    wv = nc.dram_tensor("wv", [D, H * DF], f32, kind="ExternalInput").ap()
    wo = nc.dram_tensor("wo", [H * DF, D], f32, kind="ExternalInput").ap()
    wt = nc.dram_tensor("wt", [H * DF, G], f32, kind="ExternalInput").ap()
    y = nc.dram_tensor("y", [PH, D], f32, kind="ExternalOutput").ap()

    with tile.TileContext(nc) as tc, ExitStack() as ctx:
        persist = ctx.enter_context(tc.tile_pool(name="persist", bufs=1))
        qt_sb = persist.tile([128, EC * PH], bf16, tag="qt")      # Q^T [e, p]
        kt_sb = persist.tile([128, EC * P], bf16, tag="kt")       # K^T [e, q]
        v_sb = persist.tile([128, QC * G * VW], bf16, tag="v")    # V' [q, g*65+c]
        wo_sb = persist.tile([128, EC * D], bf16, tag="wo")
        wt_sb = persist.tile([128, EC * G], f32, tag="wt")
        ocat_sb = persist.tile([128, EC * PH], bf16, tag="ocat")  # out^T [e, p]

        def cast_load(dst_tile, dram_ap, n):
            # SWDGE cast f32 DRAM -> bf16 SBUF, one DMA per tensor
            nc.gpsimd.dma_start(
                dst_tile[:].rearrange("p (c m) -> p c m", c=n),
                dram_ap.rearrange("(c p) m -> p c m", p=128))

        # ---------- phase B: staging + QKV projections ----------
        with ExitStack() as pb:
            stage = pb.enter_context(tc.tile_pool(name="stage", bufs=1))
            xt_sb = stage.tile([128, DC * P], bf16, tag="xt")
            xqt_sb = stage.tile([128, DC * PH], bf16, tag="xqt")
            wq_sb = stage.tile([128, DC * D], bf16, tag="wq")
            wk_sb = stage.tile([128, DC * D], bf16, tag="wk")
            wv_sb = stage.tile([128, DC * D], bf16, tag="wv")
            for dc in range(DC):
                nc.gpsimd.dma_start(xqt_sb[:, dc * PH:(dc + 1) * PH],
                                    xqt[dc * 128:(dc + 1) * 128, :])
                nc.gpsimd.dma_start(wq_sb[:, dc * D:(dc + 1) * D],
                                    wq[dc * 128:(dc + 1) * 128, :])
            cast_load(xt_sb, xt, DC)
            cast_load(wk_sb, wk, DC)
            cast_load(wv_sb, wv, DC)
            cast_load(wo_sb, wo, EC)
            nc.sync.dma_start(
                wt_sb[:].rearrange("p (c m) -> p c m", c=EC),
                wt.rearrange("(c p) m -> p c m", p=128))

            nc.gpsimd.memset(v_sb[:], 1.0)  # ones columns of V'

            psA = pb.enter_context(tc.tile_pool(name="psA", bufs=2, space="PSUM"))
            psB = pb.enter_context(tc.tile_pool(name="psB", bufs=4, space="PSUM"))

            # Q^T[e, p] = Wq^T x^T (query half only)
            for ec in range(EC):
                q_ps = psA.tile([128, PH], f32, tag="qps")
                for pc in range(PH // 512):
                    for dc in range(DC):
                        nc.tensor.matmul(
                            q_ps[:, pc * 512:(pc + 1) * 512],
                            lhsT=wq_sb[:, dc * D + ec * 128: dc * D + (ec + 1) * 128],
                            rhs=xqt_sb[:, dc * PH + pc * 512: dc * PH + (pc + 1) * 512],
                            start=(dc == 0), stop=(dc == DC - 1))
                nc.scalar.activation(qt_sb[:, ec * PH:(ec + 1) * PH], q_ps[:], ACT.Copy)
            # K^T[e, q] over all keys
            for ec in range(EC):
                for qn in range(P // 512):
                    k_ps = psB.tile([128, 512], f32, tag="kvps")
                    for dc in range(DC):
                        nc.tensor.matmul(
                            k_ps[:],
                            lhsT=wk_sb[:, dc * D + ec * 128: dc * D + (ec + 1) * 128],
                            rhs=xt_sb[:, dc * P + qn * 512: dc * P + (qn + 1) * 512],
                            start=(dc == 0), stop=(dc == DC - 1))
                    nc.scalar.activation(
                        kt_sb[:, ec * P + qn * 512: ec * P + (qn + 1) * 512],
                        k_ps[:], ACT.Copy)
            # V[q, e] natural layout, scattered into V' with ones columns
            for qc in range(QC):
                v_ps = psB.tile([128, 512], f32, tag="kvps")
                for dc in range(DC):
                    nc.tensor.matmul(
                        v_ps[:],
                        lhsT=xt_sb[:, dc * P + qc * 128: dc * P + (qc + 1) * 128],
                        rhs=wv_sb[:, dc * D:(dc + 1) * D],
                        start=(dc == 0), stop=(dc == DC - 1))
                dst = v_sb[:, qc * G * VW:(qc + 1) * G * VW]
                dst = dst.rearrange("p (g c) -> p g c", c=VW)[:, :, 0:DF]
                src = v_ps[:].rearrange("p (g c) -> p g c", c=DF)
                nc.vector.tensor_copy(dst, src)

        # ---------- phase C: attention main loop ----------
        with ExitStack() as pcs:
            qg_pool = pcs.enter_context(tc.tile_pool(name="qg", bufs=2))
            bias_pool = pcs.enter_context(tc.tile_pool(name="bias", bufs=4))
            exp_pool = pcs.enter_context(tc.tile_pool(name="exp", bufs=3))
            nrm_pool = pcs.enter_context(tc.tile_pool(name="nrm", bufs=2))
            s_pool = pcs.enter_context(tc.tile_pool(name="sps", bufs=3, space="PSUM"))
            o_pool = pcs.enter_context(tc.tile_pool(name="ops", bufs=1, space="PSUM"))

            for g in range(G):
                # Qg^T = Q^T * (Wtalk[g, h] / sqrt(DF)) -- folds the head mix
                qg_sb = qg_pool.tile([128, EC * PH], bf16, tag="qg")
                for ec in range(EC):
                    nc.vector.tensor_scalar_mul(
                        qg_sb[:, ec * PH:(ec + 1) * PH],
                        qt_sb[:, ec * PH:(ec + 1) * PH],
                        wt_sb[:, ec * G + g: ec * G + g + 1])
                o_ps = o_pool.tile([VW, PH], f32, tag="ops")
                for qc in range(QC):
                    b_sb = bias_pool.tile([128, PH], bf16, tag="bias")
                    nc.sync.dma_start(b_sb[:], biast[g, qc * 128:(qc + 1) * 128, :])
                    s_ps = s_pool.tile([128, PH], f32, tag="sps")
                    e_sb = exp_pool.tile([128, PH], bf16, tag="exp")
                    for ec in range(EC):
                        for pc in range(PH // 512):
                            nc.tensor.matmul(
                                s_ps[:, pc * 512:(pc + 1) * 512],
                                lhsT=kt_sb[:, ec * P + qc * 128: ec * P + (qc + 1) * 128],
                                rhs=qg_sb[:, ec * PH + pc * 512: ec * PH + (pc + 1) * 512],
                                start=(ec == 0), stop=(ec == EC - 1))
                    nc.vector.tensor_add(s_ps[:], s_ps[:], b_sb[:])
                    nc.scalar.activation(e_sb[:], s_ps[:], ACT.Exp)
                    for pc in range(PH // 512):
                        nc.tensor.matmul(
                            o_ps[:, pc * 512:(pc + 1) * 512],
                            lhsT=v_sb[:, qc * G * VW + g * VW: qc * G * VW + (g + 1) * VW],
                            rhs=e_sb[:, pc * 512:(pc + 1) * 512],
                            start=(qc == 0), stop=(qc == QC - 1))
                # normalize: out^T[df, p] / sums[p]; sums sit in partition 64
                sum_sb = nrm_pool.tile([1, PH], f32, tag="sum")
                nc.scalar.activation(sum_sb[:], o_ps[DF:DF + 1, :], ACT.Copy)
                r_sb = nrm_pool.tile([1, PH], f32, tag="r")
                nc.vector.reciprocal_approx_fast(r_sb[:], sum_sb[:])
                rb_sb = nrm_pool.tile([DF, PH], f32, tag="rb")
                nc.gpsimd.partition_broadcast(rb_sb[:], r_sb[:])
                po, fo = (g % 2) * DF, (g // 2) * PH
                nc.vector.tensor_mul(
                    ocat_sb[po:po + DF, fo:fo + PH], o_ps[0:DF, :], rb_sb[:])

        # ---------- phase D: output projection ----------
        with ExitStack() as pd:
            y_pool = pd.enter_context(tc.tile_pool(name="yps", bufs=2, space="PSUM"))
            ysb_pool = pd.enter_context(tc.tile_pool(name="ysb", bufs=2))
            for pc in range(PH // 128):
                y_ps = y_pool.tile([128, D], f32, tag="yps")
                for ec in range(EC):
                    nc.tensor.matmul(
                        y_ps[:],
                        lhsT=ocat_sb[:, ec * PH + pc * 128: ec * PH + (pc + 1) * 128],
                        rhs=wo_sb[:, ec * D:(ec + 1) * D],
                        start=(ec == 0), stop=(ec == EC - 1))
                y_sb = ysb_pool.tile([128, D], f32, tag="ysb")
                nc.scalar.activation(y_sb[:], y_ps[:], ACT.Copy)
                nc.sync.dma_start(y[pc * 128:(pc + 1) * 128, :], y_sb[:])

    nc.compile()
    return nc


def kernel(x, attn_bias, Wq, Wk, Wv, Wtalk, Wo, **trace_kwargs):
    global LAST_RESULTS
    from concourse.bass_utils import run_bass_kernel_spmd

    x = np.asarray(x, dtype=np.float32)
    attn_bias = np.asarray(attn_bias, dtype=np.float32)
    Wq = np.asarray(Wq, dtype=np.float32)
    Wk = np.asarray(Wk, dtype=np.float32)
    Wv = np.asarray(Wv, dtype=np.float32)
    Wtalk = np.asarray(Wtalk, dtype=np.float32)
    Wo = np.asarray(Wo, dtype=np.float32)

    if "nc" not in _CACHE:
        _CACHE["nc"] = _build_program()
    nc = _CACHE["nc"]

    # host-side layout prep (cheap, reused across cores)
    xts = [np.ascontiguousarray(x[b].T) for b in range(B)]                 # [D, P]
    xqts = [[np.ascontiguousarray(x[b, s * PH:(s + 1) * PH, :].T) for s in range(2)]
            for b in range(B)]                                             # [D, PH]
    import ml_dtypes
    biasts = [np.ascontiguousarray(
        attn_bias[0, :, s * PH:(s + 1) * PH, :].transpose(0, 2, 1))
        .astype(ml_dtypes.bfloat16) for s in range(2)]
    wt = np.ascontiguousarray((np.repeat(Wtalk, DF, axis=1) / np.sqrt(DF)).T
                              .astype(np.float32))                         # [512, 8]

    in_maps = []
    for c in range(N_CORES):
        b, s = c // 2, c % 2
        in_maps.append({
            "xt": xts[b], "xqt": xqts[b][s], "biast": biasts[s],
            "wq": Wq, "wk": Wk, "wv": Wv, "wo": Wo, "wt": wt,
        })

    res = run_bass_kernel_spmd(nc, in_maps, list(range(N_CORES)), **trace_kwargs)
    LAST_RESULTS = res

    out = np.empty((B, P, D), dtype=np.float32)
    for c in range(N_CORES):
        b, s = c // 2, c % 2
        out[b, s * PH:(s + 1) * PH, :] = res.results[c]["y"]
    return out
